# revision 3
# baseline (speedup 1.0000x reference)
"""Trainium2 Bass kernel for BinaryMemoryTree logits.

logits[b,k,c] = sum_{d,e} q[b,k,d] * memory[b,c,d,e] * v[b,k,e]

Data-parallel over batch B=8 -> one batch element per NeuronCore.

Per-core pipeline, per chunk of 4 query tiles (128 queries each):
  PE:   4 transposes q -> qT (PSUM);  Act: evac qT -> SBUF (bf16)
  PE:   4 matmuls ctx[k,(c,e)] = qT^T @ [M0|M1] -> PSUM fp32
  mult: greedy split:
          DVE reads ctx from PSUM directly  (w ~ 1.2us/chunk)
          or Act evacs ctx -> SBUF fp32, Pool (gpsimd) multiplies
        product lands bf16 in SBUF
  per block of 16 tiles: DVE pairwise-halving add chain (bf16 2x mode)
        128 -> 64 -> ... -> 1, last level emits fp32 logits
  one DMA store of all logits per iteration

Constants (identity, M) and all tile pools live outside the bench rep
loop so iterations pipeline cleanly.
"""

import sys

sys.path.insert(0, "/opt/trn_rl_repo")

import os as _os

import numpy as np
from concourse import bacc, bass, bass_utils, masks, mybir, tile

B = 8
L = 32768
D = 128
C = 2
P = 128

F32 = mybir.dt.float32
F32R = mybir.dt.float32r
BF16 = mybir.dt.bfloat16

MM_DTYPE = _os.environ.get("K5_MM_DTYPE", "bf16")
CTX_BUFS = int(_os.environ.get("K5_CTX_BUFS", "3"))
CHUNK_T = 4
DMA_BLK = int(_os.environ.get("K5_DMA_BLK", "1"))
# stop halving chain at this width (>1 -> finish with tensor_reduce)
RED_W = int(_os.environ.get("K5_RED_W", "64"))
# measured HW per-chunk mult costs (ns)
W_DVE_CHUNK = int(_os.environ.get("K5_W_DVE", "1190"))
W_POOL_CHUNK = int(_os.environ.get("K5_W_POOL", "2050"))
W_RED_BLK = int(_os.environ.get("K5_W_RED", "4280"))
LAST_DVE = _os.environ.get("K5_LAST_DVE", "1") == "1"
HALF_BLK = _os.environ.get("K5_HALF_BLK", "0") == "1"
V_ENG = _os.environ.get("K5_V_ENG", "sync")
# emit block B's reduction chain after chunk DELAY_CHAIN of block B+1 so
# the next block's DVE mults run first and free PSUM ctx buffers sooner
DELAY_CHAIN = int(_os.environ.get("K5_DELAY_CHAIN", "-1"))  # -1 = off

TILES = L // P
BLK_T = 16
NBLK = TILES // BLK_T
NCH = BLK_T // CHUNK_T


def _body(tc, nc, pools, consts, stage):
    iop, qtps, ctxps, workp, outp = pools
    ident, m_sb, q_view, v_view, o_view = consts
    state = {"acc_d": 0, "acc_p": 0, "pending_chain": None}

    NG = NBLK // DMA_BLK
    o_all = outp.tile([P, NBLK, BLK_T, C], F32, tag="o_all")

    for blk in range(NBLK):
        g, b = divmod(blk, DMA_BLK)
        if b == 0:
            qg_sb = iop.tile([P, DMA_BLK * BLK_T, D], F32, tag="q")
            vg_sb = iop.tile([P, DMA_BLK * BLK_T, D], F32, tag="v")
            nc.sync.dma_start(qg_sb[:], q_view[g])
            getattr(nc, V_ENG).dma_start(vg_sb[:], v_view[g])
        q_sb = qg_sb[:, b * BLK_T:(b + 1) * BLK_T, :]
        v_sb = vg_sb[:, b * BLK_T:(b + 1) * BLK_T, :]
        o_sb = o_all[:, blk]

        if stage == "dma":
            nc.vector.tensor_reduce(
                out=o_sb[:, :, 0], in_=q_sb[:],
                axis=mybir.AxisListType.X, op=mybir.AluOpType.add)
            nc.vector.tensor_reduce(
                out=o_sb[:, :, 1], in_=v_sb[:],
                axis=mybir.AxisListType.X, op=mybir.AluOpType.add)
            continue

        p_blk = workp.tile([P, BLK_T, C, D], BF16, tag="P")

        # reduction over e: bf16 pairwise-halving adds on DVE (2x mode)
        def emit_chain(t_lo, t_hi, tag_sfx="", p_blk=p_blk, o_sb=o_sb):
            red_in = p_blk[:, t_lo:t_hi]
            o_slice = o_sb[:, t_lo:t_hi]
            nt = t_hi - t_lo
            w = D
            while w > RED_W:
                w //= 2
                if w == 1:
                    nc.vector.tensor_tensor(
                        out=o_slice.unsqueeze(3), in0=red_in[:, :, :, 0:1],
                        in1=red_in[:, :, :, 1:2], op=mybir.AluOpType.add)
                else:
                    hv = workp.tile([P, nt, C, w], BF16,
                                    tag=f"hv{w}{tag_sfx}")
                    nc.vector.tensor_tensor(
                        out=hv[:], in0=red_in[:, :, :, :w],
                        in1=red_in[:, :, :, w:], op=mybir.AluOpType.add)
                    red_in = hv[:]
            if RED_W > 1:
                nc.vector.tensor_reduce(
                    out=o_slice, in_=red_in, axis=mybir.AxisListType.X,
                    op=mybir.AluOpType.add)

        for ch in range(NCH):
            sl = slice(ch * CHUNK_T, (ch + 1) * CHUNK_T)
            qT = qtps.tile([P, CHUNK_T, P], F32, tag="qT")
            for t in range(CHUNK_T):
                tt = ch * CHUNK_T + t
                nc.tensor.transpose(qT[:, t, :], q_sb[:, tt, :], ident[:])
            qT_sb = workp.tile([P, CHUNK_T, P],
                               BF16 if MM_DTYPE == "bf16" else F32R, tag="qTs")
            nc.scalar.copy(qT_sb[:], qT[:])

            ctx = ctxps.tile([P, CHUNK_T, C, D], F32, tag="ctx")
            for t in range(CHUNK_T):
                nc.tensor.matmul(
                    ctx[:, t, :, :], qT_sb[:, t, :], m_sb[:],
                    start=True, stop=True)

            if stage == "matmul":
                nc.vector.tensor_reduce(
                    out=o_sb[:, sl, :], in_=ctx[:],
                    axis=mybir.AxisListType.X, op=mybir.AluOpType.add)
                continue

            v_bc = v_sb[:, sl, :].unsqueeze(2).broadcast_to(
                [P, CHUNK_T, C, D])
            force_dve = LAST_DVE and ch == NCH - 1
            if force_dve or (
                state["acc_d"] + W_DVE_CHUNK <= state["acc_p"] + W_POOL_CHUNK
            ):
                state["acc_d"] += W_DVE_CHUNK
                nc.vector.tensor_tensor(
                    out=p_blk[:, sl], in0=ctx[:], in1=v_bc,
                    op=mybir.AluOpType.mult)
            else:
                state["acc_p"] += W_POOL_CHUNK
                ctx_sb = workp.tile([P, CHUNK_T, C, D], F32, tag="ctxs")
                nc.scalar.copy(ctx_sb[:], ctx[:])
                nc.gpsimd.tensor_tensor(
                    out=p_blk[:, sl], in0=ctx_sb[:], in1=v_bc,
                    op=mybir.AluOpType.mult)

            if HALF_BLK and stage == "full" and ch == NCH // 2 - 1:
                emit_chain(0, BLK_T // 2, "a")

            if (stage == "full" and ch == DELAY_CHAIN
                    and state["pending_chain"] is not None):
                state["pending_chain"]()
                state["pending_chain"] = None

        if stage == "matmul":
            continue

        state["acc_d"] += W_RED_BLK
        if HALF_BLK:
            emit_chain(BLK_T // 2, BLK_T, "b")
        elif DELAY_CHAIN >= 0:
            state["pending_chain"] = lambda f=emit_chain: f(0, BLK_T)
        else:
            emit_chain(0, BLK_T)

    if state["pending_chain"] is not None:
        state["pending_chain"]()
        state["pending_chain"] = None

    nc.sync.dma_start(o_view, o_all[:].rearrange("p n t c -> p (n t) c"))


_NC_CACHE = {}


def _build(reps=1, stage="full"):
    key = ("nc", reps, stage, MM_DTYPE, DMA_BLK, CTX_BUFS, RED_W,
           W_DVE_CHUNK, W_POOL_CHUNK, W_RED_BLK, LAST_DVE, HALF_BLK, V_ENG, DELAY_CHAIN)
    if key in _NC_CACHE:
        return _NC_CACHE[key]
    nc = bacc.Bacc("TRN2", target_bir_lowering=False, debug=False)
    qd = nc.dram_tensor("q", (L, D), F32, kind="ExternalInput")
    vd = nc.dram_tensor("v", (L, D), F32, kind="ExternalInput")
    md = nc.dram_tensor("m", (C, D, D), F32, kind="ExternalInput")
    od = nc.dram_tensor("o", (L, C), F32, kind="ExternalOutput")
    with tile.TileContext(nc) as tc:
        with tc.tile_pool(name="const", bufs=1) as constp, \
             tc.tile_pool(name="io", bufs=2) as iop, \
             tc.tile_pool(name="qt_ps", bufs=2, space="PSUM") as qtps, \
             tc.tile_pool(name="ctx_ps", bufs=CTX_BUFS, space="PSUM") as ctxps, \
             tc.tile_pool(name="work", bufs=2) as workp, \
             tc.tile_pool(name="out", bufs=2) as outp:

            ident = constp.tile([P, P], F32)
            masks.make_identity(nc, ident[:])
            m_raw = constp.tile([P, C, D], F32)
            nc.sync.dma_start(m_raw[:], md.ap().transpose([1, 0, 2]))
            mm_dt = {"f32r": F32R, "bf16": BF16}[MM_DTYPE]
            m_sb = constp.tile([P, C, D], mm_dt)
            nc.scalar.copy(m_sb[:], m_raw[:])

            NG = NBLK // DMA_BLK
            q_view = qd.ap().rearrange(
                "(p g t) d -> g p t d", p=P, g=NG, t=DMA_BLK * BLK_T)
            v_view = vd.ap().rearrange(
                "(p g t) d -> g p t d", p=P, g=NG, t=DMA_BLK * BLK_T)
            o_view = od.ap().rearrange("(p j) c -> p j c", p=P)

            pools = (iop, qtps, ctxps, workp, outp)
            consts = (ident, m_sb, q_view, v_view, o_view)
            if reps == 1:
                _body(tc, nc, pools, consts, stage)
            else:
                with tc.For_i(0, reps, 1):
                    _body(tc, nc, pools, consts, stage)
    nc.compile()
    _NC_CACHE[key] = nc
    return nc


def kernel(q, v, memory, _trace=False, _reps=1, _stage="full"):
    nc = _build(_reps, _stage)
    q = np.asarray(q, dtype=np.float32)
    v = np.asarray(v, dtype=np.float32)
    memory = np.asarray(memory, dtype=np.float32)
    in_maps = [
        {
            "q": np.ascontiguousarray(q[b]),
            "v": np.ascontiguousarray(v[b]),
            "m": np.ascontiguousarray(memory[b]),
        }
        for b in range(B)
    ]
    res = bass_utils.run_bass_kernel_spmd(
        nc, in_maps, core_ids=list(range(B)), trace=_trace
    )
    out = np.stack([res.results[b]["o"] for b in range(B)])
    if _trace:
        kernel.last_result = res
    return out


# revision 6
# speedup vs baseline: 1.5064x; 1.5064x over previous
"""Trainium2 Bass kernel for BinaryMemoryTree logits.

logits[b,k,c] = sum_{d,e} q[b,k,d] * memory[b,c,d,e] * v[b,k,e]

Data-parallel over batch B=8 -> one batch element per NeuronCore.

Per-core pipeline, per chunk of 4 query tiles (128 queries each):
  PE:   4 transposes q -> qT (PSUM);  Act: evac qT -> SBUF (bf16)
  PE:   4 matmuls ctx[k,(c,e)] = qT^T @ [M0|M1] -> PSUM fp32
  mult: greedy split:
          DVE reads ctx from PSUM directly  (w ~ 1.2us/chunk)
          or Act evacs ctx -> SBUF fp32, Pool (gpsimd) multiplies
        product lands bf16 in SBUF
  per block of 16 tiles: DVE pairwise-halving add chain (bf16 2x mode)
        128 -> 64 -> ... -> 1, last level emits fp32 logits
  one DMA store of all logits per iteration

Constants (identity, M) and all tile pools live outside the bench rep
loop so iterations pipeline cleanly.
"""

import sys

sys.path.insert(0, "/opt/trn_rl_repo")

import os as _os

import numpy as np
from concourse import bacc, bass, bass_utils, masks, mybir, tile

B = 8
L = 32768
D = 128
C = 2
P = 128

F32 = mybir.dt.float32
F32R = mybir.dt.float32r
BF16 = mybir.dt.bfloat16

MM_DTYPE = _os.environ.get("K5_MM_DTYPE", "bf16")
CTX_BUFS = int(_os.environ.get("K5_CTX_BUFS", "3"))
IO_BUFS = int(_os.environ.get("K5_IO_BUFS", "3"))
WORK_BUFS = int(_os.environ.get("K5_WORK_BUFS", "3"))
CHUNK_T = 4
DMA_BLK = int(_os.environ.get("K5_DMA_BLK", "1"))
# stop halving chain at this width (>1 -> finish with tensor_reduce)
RED_W = int(_os.environ.get("K5_RED_W", "64"))
# measured HW per-chunk mult costs (ns)
W_DVE_CHUNK = int(_os.environ.get("K5_W_DVE", "1190"))
W_POOL_CHUNK = int(_os.environ.get("K5_W_POOL", "2050"))
W_RED_BLK = int(_os.environ.get("K5_W_RED", "4280"))
LAST_DVE = _os.environ.get("K5_LAST_DVE", "1") == "1"
HALF_BLK = _os.environ.get("K5_HALF_BLK", "0") == "1"
V_ENG = _os.environ.get("K5_V_ENG", "sync")
# emit block B's reduction chain after chunk DELAY_CHAIN of block B+1 so
# the next block's DVE mults run first and free PSUM ctx buffers sooner
DELAY_CHAIN = int(_os.environ.get("K5_DELAY_CHAIN", "-1"))  # -1 = off

TILES = L // P
BLK_T = int(_os.environ.get("K5_BLK_T", "16"))
NBLK = TILES // BLK_T
NCH = BLK_T // CHUNK_T


def _body(tc, nc, pools, consts, stage):
    iop, qtps, ctxps, workp, outp = pools
    ident, m_sb, q_view, v_view, o_view = consts
    state = {"acc_d": 0, "acc_p": 0, "pending_chain": None}

    NG = NBLK // DMA_BLK
    o_all = outp.tile([P, NBLK, BLK_T, C], F32, tag="o_all")

    for blk in range(NBLK):
        g, b = divmod(blk, DMA_BLK)
        if b == 0:
            qg_sb = iop.tile([P, DMA_BLK * BLK_T, D], F32, tag="q")
            vg_sb = iop.tile([P, DMA_BLK * BLK_T, D], F32, tag="v")
            nc.sync.dma_start(qg_sb[:], q_view[g])
            getattr(nc, V_ENG).dma_start(vg_sb[:], v_view[g])
        q_sb = qg_sb[:, b * BLK_T:(b + 1) * BLK_T, :]
        v_sb = vg_sb[:, b * BLK_T:(b + 1) * BLK_T, :]
        o_sb = o_all[:, blk]

        if stage == "dma":
            nc.vector.tensor_reduce(
                out=o_sb[:, :, 0], in_=q_sb[:],
                axis=mybir.AxisListType.X, op=mybir.AluOpType.add)
            nc.vector.tensor_reduce(
                out=o_sb[:, :, 1], in_=v_sb[:],
                axis=mybir.AxisListType.X, op=mybir.AluOpType.add)
            continue

        p_blk = workp.tile([P, BLK_T, C, D], BF16, tag="P")

        # reduction over e: bf16 pairwise-halving adds on DVE (2x mode)
        def emit_chain(t_lo, t_hi, tag_sfx="", p_blk=p_blk, o_sb=o_sb):
            red_in = p_blk[:, t_lo:t_hi]
            o_slice = o_sb[:, t_lo:t_hi]
            nt = t_hi - t_lo
            w = D
            while w > RED_W:
                w //= 2
                if w == 1:
                    nc.vector.tensor_tensor(
                        out=o_slice.unsqueeze(3), in0=red_in[:, :, :, 0:1],
                        in1=red_in[:, :, :, 1:2], op=mybir.AluOpType.add)
                else:
                    hv = workp.tile([P, nt, C, w], BF16,
                                    tag=f"hv{w}{tag_sfx}")
                    nc.vector.tensor_tensor(
                        out=hv[:], in0=red_in[:, :, :, :w],
                        in1=red_in[:, :, :, w:], op=mybir.AluOpType.add)
                    red_in = hv[:]
            if RED_W > 1:
                nc.vector.tensor_reduce(
                    out=o_slice, in_=red_in, axis=mybir.AxisListType.X,
                    op=mybir.AluOpType.add)

        for ch in range(NCH):
            sl = slice(ch * CHUNK_T, (ch + 1) * CHUNK_T)
            qT = qtps.tile([P, CHUNK_T, P], F32, tag="qT")
            for t in range(CHUNK_T):
                tt = ch * CHUNK_T + t
                nc.tensor.transpose(qT[:, t, :], q_sb[:, tt, :], ident[:])
            qT_sb = workp.tile([P, CHUNK_T, P],
                               BF16 if MM_DTYPE == "bf16" else F32R, tag="qTs")
            nc.scalar.copy(qT_sb[:], qT[:])

            ctx = ctxps.tile([P, CHUNK_T, C, D], F32, tag="ctx")
            for t in range(CHUNK_T):
                nc.tensor.matmul(
                    ctx[:, t, :, :], qT_sb[:, t, :], m_sb[:],
                    start=True, stop=True)

            if stage == "matmul":
                nc.vector.tensor_reduce(
                    out=o_sb[:, sl, :], in_=ctx[:],
                    axis=mybir.AxisListType.X, op=mybir.AluOpType.add)
                continue

            v_bc = v_sb[:, sl, :].unsqueeze(2).broadcast_to(
                [P, CHUNK_T, C, D])
            force_dve = LAST_DVE and ch == NCH - 1
            if force_dve or (
                state["acc_d"] + W_DVE_CHUNK <= state["acc_p"] + W_POOL_CHUNK
            ):
                state["acc_d"] += W_DVE_CHUNK
                nc.vector.tensor_tensor(
                    out=p_blk[:, sl], in0=ctx[:], in1=v_bc,
                    op=mybir.AluOpType.mult)
            else:
                state["acc_p"] += W_POOL_CHUNK
                ctx_sb = workp.tile([P, CHUNK_T, C, D], F32, tag="ctxs")
                nc.scalar.copy(ctx_sb[:], ctx[:])
                nc.gpsimd.tensor_tensor(
                    out=p_blk[:, sl], in0=ctx_sb[:], in1=v_bc,
                    op=mybir.AluOpType.mult)

            if HALF_BLK and stage == "full" and ch == NCH // 2 - 1:
                emit_chain(0, BLK_T // 2, "a")

            if (stage == "full" and ch == DELAY_CHAIN
                    and state["pending_chain"] is not None):
                state["pending_chain"]()
                state["pending_chain"] = None

        if stage == "matmul":
            continue

        state["acc_d"] += W_RED_BLK
        if HALF_BLK:
            emit_chain(BLK_T // 2, BLK_T, "b")
        elif DELAY_CHAIN >= 0:
            state["pending_chain"] = lambda f=emit_chain: f(0, BLK_T)
        else:
            emit_chain(0, BLK_T)

    if state["pending_chain"] is not None:
        state["pending_chain"]()
        state["pending_chain"] = None

    nc.sync.dma_start(o_view, o_all[:].rearrange("p n t c -> p (n t) c"))


_NC_CACHE = {}


def _build(reps=1, stage="full"):
    key = ("nc", reps, stage, MM_DTYPE, DMA_BLK, CTX_BUFS, IO_BUFS, WORK_BUFS, RED_W, BLK_T,
           W_DVE_CHUNK, W_POOL_CHUNK, W_RED_BLK, LAST_DVE, HALF_BLK, V_ENG, DELAY_CHAIN)
    if key in _NC_CACHE:
        return _NC_CACHE[key]
    nc = bacc.Bacc("TRN2", target_bir_lowering=False, debug=False)
    qd = nc.dram_tensor("q", (L, D), F32, kind="ExternalInput")
    vd = nc.dram_tensor("v", (L, D), F32, kind="ExternalInput")
    md = nc.dram_tensor("m", (C, D, D), F32, kind="ExternalInput")
    od = nc.dram_tensor("o", (L, C), F32, kind="ExternalOutput")
    with tile.TileContext(nc) as tc:
        with tc.tile_pool(name="const", bufs=1) as constp, \
             tc.tile_pool(name="io", bufs=IO_BUFS) as iop, \
             tc.tile_pool(name="qt_ps", bufs=2, space="PSUM") as qtps, \
             tc.tile_pool(name="ctx_ps", bufs=CTX_BUFS, space="PSUM") as ctxps, \
             tc.tile_pool(name="work", bufs=WORK_BUFS) as workp, \
             tc.tile_pool(name="out", bufs=2) as outp:

            ident = constp.tile([P, P], F32)
            masks.make_identity(nc, ident[:])
            m_raw = constp.tile([P, C, D], F32)
            nc.sync.dma_start(m_raw[:], md.ap().transpose([1, 0, 2]))
            mm_dt = {"f32r": F32R, "bf16": BF16}[MM_DTYPE]
            m_sb = constp.tile([P, C, D], mm_dt)
            nc.scalar.copy(m_sb[:], m_raw[:])

            NG = NBLK // DMA_BLK
            q_view = qd.ap().rearrange(
                "(p g t) d -> g p t d", p=P, g=NG, t=DMA_BLK * BLK_T)
            v_view = vd.ap().rearrange(
                "(p g t) d -> g p t d", p=P, g=NG, t=DMA_BLK * BLK_T)
            o_view = od.ap().rearrange("(p j) c -> p j c", p=P)

            pools = (iop, qtps, ctxps, workp, outp)
            consts = (ident, m_sb, q_view, v_view, o_view)
            if reps == 1:
                _body(tc, nc, pools, consts, stage)
            else:
                with tc.For_i(0, reps, 1):
                    _body(tc, nc, pools, consts, stage)
    nc.compile()
    _NC_CACHE[key] = nc
    return nc


def kernel(q, v, memory, _trace=False, _reps=1, _stage="full"):
    nc = _build(_reps, _stage)
    q = np.asarray(q, dtype=np.float32)
    v = np.asarray(v, dtype=np.float32)
    memory = np.asarray(memory, dtype=np.float32)
    in_maps = [
        {
            "q": np.ascontiguousarray(q[b]),
            "v": np.ascontiguousarray(v[b]),
            "m": np.ascontiguousarray(memory[b]),
        }
        for b in range(B)
    ]
    res = bass_utils.run_bass_kernel_spmd(
        nc, in_maps, core_ids=list(range(B)), trace=_trace
    )
    out = np.stack([res.results[b]["o"] for b in range(B)])
    if _trace:
        kernel.last_result = res
    return out
